# revision 17
# baseline (speedup 1.0000x reference)
"""Trainium2 Bass kernel for nn_AttnDownBlock (diffusion-UNet AttnDownBlock2D).

Network (per sample): 2 x [ResNetBlock -> AttnBlock] -> strided 3x3 downsample conv.
Sharding: data-parallel over batch — 8 samples, one per NeuronCore. No collectives.

Layouts (per core):
  activations: [C partitions (x 128-tiles), H*W free] ("transposed" conv layout)
  convs: 9-tap accumulated matmuls over a zero-padded [C, 34, 34] buffer
  attention: scores^T per head ([sk, sq]); AV via col-placed M=64 matmuls; softmax
  sums via a ones-column appended to v; normalization via ones-matmul broadcast.
Weights are pre-transposed on the host so no on-device transposes are needed.
dtypes: conv path fp32r (full-rate matmul, ~2e-4), attention path bf16.
"""

import os
import numpy as np

import concourse.bacc as bacc
import concourse.bass as bass
import concourse.tile as tile
from concourse import mybir
from concourse import bass_utils

f32 = mybir.dt.float32
f32r = mybir.dt.float32r
bf16 = mybir.dt.bfloat16
AF = mybir.ActivationFunctionType
OP = mybir.AluOpType

B = 8
C1, C2 = 256, 512
HW = 32
S = HW * HW          # 1024
PW = HW + 2          # 34
PA = PW * PW         # 1156
G = 32               # norm groups
T = 512              # temb dim
HEADS, D = 8, 64
EPS = 1e-6
NT1, NT2 = C1 // 128, C2 // 128   # 2, 4


def _ind_consts():
    """Group-indicator matrices for GroupNorm cross-partition aggregation."""
    out = {}
    for gs in (8, 16):
        ngt = 128 // gs
        ind1 = np.zeros((128, ngt), np.float32)
        ind2 = np.zeros((ngt, 128), np.float32)
        for c in range(128):
            ind1[c, c // gs] = 1.0 / gs
            ind2[c // gs, c] = 1.0
        out[gs] = (ind1, ind2)
    return out


def build(debug=False):
    nc = bacc.Bacc("TRN2", target_bir_lowering=False)
    dr = {}

    def din(name, shape, dtype=f32r):
        dr[name] = nc.dram_tensor(name, list(shape), dtype, kind="ExternalInput")
        return dr[name]

    # ---- inputs ----
    din("x", (C1, S))
    din("temb", (128, 4), f32)
    for i in range(2):
        cin = C1 if i == 0 else C2
        nt = cin // 128
        din(f"rb{i}_g1s", (128, nt), f32)
        din(f"rb{i}_g1b", (128, nt), f32)
        din(f"rb{i}_c1wT", (9, cin, C2))
        din(f"rb{i}_cb1", (128, 4), f32)       # c1b + tb
        din(f"rb{i}_twT", (T, C2), bf16)
        din(f"rb{i}_g2s", (128, 4), f32)
        din(f"rb{i}_g2b", (128, 4), f32)
        din(f"rb{i}_c2wT", (9, C2, C2))
        din(f"rb{i}_cb2", (128, 4), f32)       # c2b (+ skb for rb0)
        din(f"at{i}_ns", (128, 4), f32)
        din(f"at{i}_nb", (128, 4), f32)
        for w in ("qwT", "kwT", "vwT", "pwT"):
            din(f"at{i}_{w}", (C2, C2), bf16)
        din(f"at{i}_qb", (128, 4), f32)
        din(f"at{i}_kb", (128, 4), f32)
        din(f"at{i}_vb", (1, C2), bf16)
        din(f"at{i}_pb", (128, 4), f32)
    din("sk_wT", (C1, C2))
    din("ds_wT", (9, C2, C2))
    din("ds_b", (128, 4), f32)

    out_d = nc.dram_tensor("out", [C2, 256], f32, kind="ExternalOutput")

    dbg = {}
    if debug:
        for nm, sh in [("hpre0", (C2, S)), ("y0", (C2, S)), ("h0", (C2, S)),
                       ("attn0", (C2, S)), ("y20", (C2, S)), ("y21", (C2, S))]:
            dbg[nm] = nc.dram_tensor("dbg_" + nm, list(sh), f32, kind="ExternalOutput")

    # ---- inline constants ----
    inds = _ind_consts()
    ones_c = nc.inline_tensor(np.ones((33, 128), np.float32), name="ones_c")
    ident_c = nc.inline_tensor(np.eye(128, dtype=np.float32), name="ident_c")
    zeros_c = nc.inline_tensor(np.zeros((1, PA), np.float32), name="zeros_c")
    ind_c = {gs: (nc.inline_tensor(inds[gs][0], name=f"ind1_{gs}"),
                  nc.inline_tensor(inds[gs][1], name=f"ind2_{gs}"))
             for gs in (8, 16)}

    with tile.TileContext(nc) as tc:
      with nc.allow_low_precision(reason="fp32r/bf16 matmul pipeline by design"):
        with (
            tc.tile_pool(name="cst", bufs=1) as cst,
            tc.tile_pool(name="wp", bufs=2) as wp,
            tc.tile_pool(name="wq", bufs=1) as wq,
            tc.tile_pool(name="act", bufs=1) as ap_,
            tc.tile_pool(name="prp", bufs=4) as prp,
            tc.tile_pool(name="sm", bufs=2) as sm,
            tc.tile_pool(name="ps", bufs=8, space="PSUM") as ps,
        ):
            # ---------- constants into SBUF ----------
            ones_sb = cst.tile([33, 128], f32r, tag="ones")
            nc.sync.dma_start(ones_sb[:], ones_c[:].bitcast(f32r))
            ones_bf = cst.tile([1, 128], bf16, tag="ones_bf")
            nc.vector.memset(ones_bf[:], 1.0)
            ident_sb = cst.tile([128, 128], f32r, tag="ident")
            nc.sync.dma_start(ident_sb[:], ident_c[:].bitcast(f32r))
            ind_sb = {}
            for gs in (8, 16):
                ngt = 128 // gs
                i1 = cst.tile([128, ngt], f32, tag=f"i1_{gs}")
                nc.sync.dma_start(i1[:], ind_c[gs][0][:])
                i2 = cst.tile([ngt, 128], f32, tag=f"i2_{gs}")
                nc.sync.dma_start(i2[:], ind_c[gs][1][:])
                ind_sb[gs] = (i1, i2)

            _uid = [0]

            def _nm(base):
                _uid[0] += 1
                return f"{base}_{_uid[0]}"

            def mm_ps(shape=(128, 512), dtype=f32):
                return ps.tile(list(shape), dtype, tag="mm", name=_nm("mm"))

            eps_sb = cst.tile([128, 1], f32, tag="eps")
            nc.vector.memset(eps_sb[:], EPS)

            # ---------- helpers ----------
            def load_w4(dram_ap, tag, dtype=f32r, kt=4):
                """[kt*128, 512] DRAM -> [128, kt, 512] SBUF."""
                t = wq.tile([128, kt, 512], dtype, tag=tag, name=_nm(tag))
                nc.sync.dma_start(t[:], dram_ap.rearrange("(a p) n -> p a n", p=128))
                return t

            def load_col(dram, tag, pool=sm, dtype=f32):
                t = pool.tile([128, 4], dtype, tag=tag, name=_nm(tag))
                nc.sync.dma_start(t[:], dr[dram][:])
                return t

            def groupnorm(src, nt, gs, sname, bname, tag):
                """src: list of nt [128,1024] tiles (f32r). Returns (A, Bb) [128,nt] f32."""
                ngt = 128 // gs
                i1, i2 = ind_sb[gs]
                gam = sm.tile([128, nt], f32, tag=f"gam{tag}")
                nc.sync.dma_start(gam[:], dr[sname][:])
                bet = sm.tile([128, nt], f32, tag=f"bet{tag}")
                nc.sync.dma_start(bet[:], dr[bname][:])

                stats = sm.tile([128, nt, 2], f32, tag=f"st{tag}")
                for t in range(nt):
                    bn = sm.tile([128, 2, 6], f32, tag=f"bn{tag}")
                    nc.vector.bn_stats(bn[:, 0, :], src[t][:, 0:512].bitcast(f32))
                    nc.vector.bn_stats(bn[:, 1, :], src[t][:, 512:1024].bitcast(f32))
                    nc.vector.bn_aggr(stats[:, t, :], bn[:])
                # e2 = var + mean^2 (per channel)
                msq = sm.tile([128, nt], f32, tag=f"msq{tag}")
                nc.scalar.square(msq[:], stats[:, :, 0])
                nc.vector.tensor_tensor(out=stats[:, :, 1], in0=stats[:, :, 1],
                                        in1=msq[:], op=OP.add)
                # per-group aggregation via fp32 indicator matmuls
                gst_ps = mm_ps((ngt, nt, 2))
                for t in range(nt):
                    nc.tensor.matmul(gst_ps[:, t, :], i1[:, :], stats[:, t, :],
                                     start=True, stop=True, skip_group_check=True)
                gst = sm.tile([ngt, nt, 2], f32, tag=f"gst{tag}")
                nc.vector.tensor_copy(gst[:], gst_ps[:])
                chst = mm_ps((128, nt, 2))
                for t in range(nt):
                    nc.tensor.matmul(chst[:, t, :], i2[:, :], gst[:, t, :],
                                     start=True, stop=True, skip_group_check=True)
                m2 = sm.tile([128, nt], f32, tag=f"m2{tag}")
                nc.scalar.square(m2[:], chst[:, :, 0])
                var = sm.tile([128, nt], f32, tag=f"var{tag}")
                nc.vector.tensor_tensor(out=var[:], in0=chst[:, :, 1], in1=m2[:],
                                        op=OP.subtract)
                sd = sm.tile([128, nt], f32, tag=f"sd{tag}")
                nc.scalar.activation(sd[:], var[:], AF.Sqrt, bias=eps_sb[:], scale=1.0)
                rsd = sm.tile([128, nt], f32, tag=f"rsd{tag}")
                nc.vector.reciprocal(rsd[:], sd[:])
                A = sm.tile([128, nt], f32, tag=f"A{tag}")
                nc.vector.tensor_tensor(out=A[:], in0=rsd[:], in1=gam[:], op=OP.mult)
                mA = sm.tile([128, nt], f32, tag=f"mA{tag}")
                nc.vector.tensor_tensor(out=mA[:], in0=chst[:, :, 0], in1=A[:], op=OP.mult)
                Bb = sm.tile([128, nt], f32, tag=f"B{tag}")
                nc.vector.tensor_tensor(out=Bb[:], in0=bet[:], in1=mA[:], op=OP.subtract)
                return A, Bb

            def make_pad(nt, dtype=f32r):
                """nt padded [128, 34*34] tiles with zeroed borders (DMA from zeros const)."""
                pads = []
                for t in range(nt):
                    p = ap_.tile([128, PA], dtype, tag=f"pad{t}", name=_nm(f"pad{t}"))
                    # top+bottom rows
                    dst = bass.AP(tensor=p.tensor, offset=p.offset,
                                  ap=[p.ap[0], [33 * PW, 2], [1, PW]])
                    src = bass.AP(tensor=zeros_c, offset=0,
                                  ap=[[0, 128], [PW, 2], [1, PW]])
                    nc.sync.dma_start(dst, src.bitcast(dtype))
                    # left + right cols of rows 1..32 (separate DMAs; the AP
                    # balancer cannot handle the fused 3-dim form)
                    for xoff in (0, 33):
                        dst = bass.AP(tensor=p.tensor, offset=p.offset + PW + xoff,
                                      ap=[p.ap[0], [PW, 32]])
                        src = bass.AP(tensor=zeros_c, offset=0,
                                      ap=[[0, 128], [1, 32]])
                        nc.sync.dma_start(dst, src.bitcast(dtype))
                    pads.append(p)
                return pads

            def pad_interior(p):
                return bass.AP(tensor=p.tensor, offset=p.offset + PW + 1,
                               ap=[p.ap[0], [PW, 32], [1, 32]])

            def conv_rhs(p, ky, kx, sl, stride=1):
                n = 16
                off = (16 * sl * stride + ky) * PW + kx
                return bass.AP(tensor=p.tensor, offset=p.offset + off,
                               ap=[p.ap[0], [PW * stride, n], [stride, 16 if stride == 2 else 32]])

            def conv3x3(pads, cin_t, wT_name, extra_mms=None, stride=1):
                """Returns list of psum tiles. extra_mms: fn(co, sl, start, stop) emitting
                additional accumulation matmuls (skip conv / residual) after the taps."""
                slabs = 2 if stride == 1 else 1
                nfree = 512 if stride == 1 else 256
                pst = [[mm_ps((128, nfree)) for _ in range(slabs)] for _ in range(4)]
                for tap in range(9):
                    ky, kx = tap // 3, tap % 3
                    wtap = wp.tile([128, cin_t, 512], f32r, tag="wtap", name=_nm("wtap"))
                    nc.sync.dma_start(
                        wtap[:], dr[wT_name][tap].rearrange("(a p) n -> p a n", p=128))
                    for co in range(4):
                        for sl in range(slabs):
                            for ci in range(cin_t):
                                last = (tap == 8 and ci == cin_t - 1
                                        and extra_mms is None)
                                nc.tensor.matmul(
                                    pst[co][sl],
                                    wtap[:, ci, co * 128:(co + 1) * 128],
                                    conv_rhs(pads[ci], ky, kx, sl, stride),
                                    start=(tap == 0 and ci == 0), stop=last,
                                    skip_group_check=True)
                if extra_mms is not None:
                    for co in range(4):
                        for sl in range(slabs):
                            extra_mms(pst, co, sl)
                return pst

            # ---------- load x ----------
            xt = [ap_.tile([128, S], f32r, tag=f"tA{t}", name=_nm(f"tA{t}")) for t in range(NT1)]
            for t in range(NT1):
                nc.sync.dma_start(xt[t][:], dr["x"][t * 128:(t + 1) * 128, :])

            # ---------- temb: silu + projections for both blocks ----------
            t_sb = sm.tile([128, 4], f32, tag="temb")
            nc.sync.dma_start(t_sb[:], dr["temb"][:])
            st_sb = sm.tile([128, 4], bf16, tag="stemb")
            nc.scalar.activation(st_sb[:], t_sb[:], AF.Silu)
            b1_eff = []
            for i in range(2):
                twT = load_w4(dr[f"rb{i}_twT"][:], tag="twT", dtype=bf16)
                tp = mm_ps((128, 4))
                for co in range(4):
                    for kt in range(4):
                        nc.tensor.matmul(tp[:, co:co + 1],
                                         twT[:, kt, co * 128:(co + 1) * 128],
                                         st_sb[:, kt:kt + 1],
                                         start=(kt == 0), stop=(kt == 3),
                                         skip_group_check=True)
                cb1 = load_col(f"rb{i}_cb1", tag="cb1")
                be = sm.tile([128, 4], f32, tag=f"b1eff{i}")
                nc.vector.tensor_tensor(out=be[:], in0=tp[:], in1=cb1[:], op=OP.add)
                b1_eff.append(be)

            # ---------- block loop ----------
            cur = xt          # current block input tiles (f32r, [128, 1024] each)
            for blk in range(2):
                cin = C1 if blk == 0 else C2
                cin_t = cin // 128
                gs1 = cin // G        # channels per group: 8 (C=256) or 16 (C=512)
                A1, B1 = groupnorm(cur, cin_t, gs1, f"rb{blk}_g1s", f"rb{blk}_g1b", "g1")
                pads = make_pad(cin_t)
                for t in range(cin_t):
                    nc.scalar.activation(pad_interior(pads[t]), cur[t][:].bitcast(f32),
                                         AF.Silu, bias=B1[:, t:t + 1], scale=A1[:, t:t + 1])

                # conv1 (cin -> 512), bias (c1b + tb + temb proj) on copy-out
                pst = conv3x3(pads, cin_t, f"rb{blk}_c1wT")
                hpre = [ap_.tile([128, S], f32, tag=f"hpre{t}", name=_nm(f"hpre{t}")) for t in range(4)]
                for co in range(4):
                    for sl in range(2):
                        nc.scalar.activation(hpre[co][:, sl * 512:(sl + 1) * 512],
                                             pst[co][sl], AF.Identity,
                                             bias=b1_eff[blk][:, co:co + 1], scale=1.0)
                if debug and blk == 0:
                    for co in range(4):
                        nc.sync.dma_start(dbg["hpre0"][co * 128:(co + 1) * 128, :], hpre[co][:])

                # GN2 + silu -> pad2
                A2, B2 = groupnorm(hpre, 4, 16, f"rb{blk}_g2s", f"rb{blk}_g2b", "g2")
                pads2 = make_pad(4)
                for t in range(4):
                    nc.scalar.activation(pad_interior(pads2[t]), hpre[t][:],
                                         AF.Silu, bias=B2[:, t:t + 1], scale=A2[:, t:t + 1])

                # conv2 + skip (blk0) / + residual (blk1)
                if blk == 0:
                    skw = wq.tile([128, 2, 512], f32r, tag="skw")
                    nc.sync.dma_start(
                        skw[:], dr["sk_wT"][:].rearrange("(a p) n -> p a n", p=128))

                    def extra0(pst, co, sl):
                        for ci in range(NT1):
                            nc.tensor.matmul(
                                pst[co][sl], skw[:, ci, co * 128:(co + 1) * 128],
                                cur[ci][:, sl * 512:(sl + 1) * 512],
                                start=False, stop=(ci == NT1 - 1),
                                skip_group_check=True)
                    extra = extra0
                else:
                    def extra1(pst, co, sl):
                        nc.tensor.matmul(pst[co][sl], ident_sb[:],
                                         cur[co][:, sl * 512:(sl + 1) * 512],
                                         start=False, stop=True,
                                         skip_group_check=True)
                    extra = extra1
                pst = conv3x3(pads2, 4, f"rb{blk}_c2wT", extra_mms=extra)
                cb2 = load_col(f"rb{blk}_cb2", tag="cb2")
                ytag = "tB" if blk == 0 else "tC"
                y = [ap_.tile([128, S], f32r, tag=f"{ytag}{t}", name=_nm(f"{ytag}{t}")) for t in range(4)]
                for co in range(4):
                    for sl in range(2):
                        nc.scalar.activation(y[co][:, sl * 512:(sl + 1) * 512],
                                             pst[co][sl], AF.Identity,
                                             bias=cb2[:, co:co + 1], scale=1.0)
                if debug and blk == 0:
                    for co in range(4):
                        nc.sync.dma_start(dbg["y0"][co * 128:(co + 1) * 128, :],
                                          y[co][:].bitcast(f32))

                # ---------- attention ----------
                A3, B3 = groupnorm(y, 4, 16, f"at{blk}_ns", f"at{blk}_nb", "g3")
                h = [ap_.tile([128, S], bf16, tag=f"h{t}", name=_nm(f"h{t}")) for t in range(4)]
                for t in range(4):
                    nc.scalar.activation(h[t][:], y[t][:].bitcast(f32),
                                         AF.Identity, bias=B3[:, t:t + 1],
                                         scale=A3[:, t:t + 1])
                if debug and blk == 0:
                    for t in range(4):
                        nc.sync.dma_start(dbg["h0"][t * 128:(t + 1) * 128, :], h[t][:])

                qwT = load_w4(dr[f"at{blk}_qwT"][:], tag="qwT", dtype=bf16)
                kwT = load_w4(dr[f"at{blk}_kwT"][:], tag="kwT", dtype=bf16)
                vwT = load_w4(dr[f"at{blk}_vwT"][:], tag="vwT", dtype=bf16)
                qb = load_col(f"at{blk}_qb", tag="qb")
                kb = load_col(f"at{blk}_kb", tag="kb")
                vb_sb = sm.tile([1, 512], bf16, tag="vb")
                nc.sync.dma_start(vb_sb[:], dr[f"at{blk}_vb"][:])

                qT = [ap_.tile([128, S], bf16, tag=f"q{t}", name=_nm(f"q{t}")) for t in range(4)]
                kT = [ap_.tile([128, S], bf16, tag=f"k{t}", name=_nm(f"k{t}")) for t in range(4)]
                for wsb, dst, bcol in ((qwT, qT, qb), (kwT, kT, kb)):
                    for co in range(4):
                        for sn in range(2):
                            qp = mm_ps()
                            for ci in range(4):
                                nc.tensor.matmul(qp[:],
                                                 wsb[:, ci, co * 128:(co + 1) * 128],
                                                 h[ci][:, sn * 512:(sn + 1) * 512],
                                                 start=(ci == 0), stop=(ci == 3))
                            nc.scalar.activation(dst[co][:, sn * 512:(sn + 1) * 512],
                                                 qp[:], AF.Identity,
                                                 bias=bcol[:, co:co + 1], scale=1.0)
                # v in [s, c] layout, augmented with a ones column per head
                vaug = [ap_.tile([128, HEADS, 65], bf16, tag=f"va{st}", name=_nm(f"va{st}")) for st in range(8)]
                for st in range(8):
                    vp = mm_ps()
                    for ci in range(4):
                        nc.tensor.matmul(vp[:], h[ci][:, st * 128:(st + 1) * 128],
                                         vwT[:, ci, :], start=(ci == 0), stop=False,
                                         skip_group_check=True)
                    nc.tensor.matmul(vp[:], ones_bf[0:1, :], vb_sb[:],
                                     start=False, stop=True,
                                     skip_group_check=True)
                    nc.scalar.copy(vaug[st][:, :, 0:64],
                                   vp[:].rearrange("p (h d) -> p h d", d=64))
                    nc.vector.memset(vaug[st][:, :, 64:65], 1.0)

                attn = [ap_.tile([128, S], bf16, tag=f"at{t}", name=_nm(f"at{t}")) for t in range(4)]
                for pr in range(4):          # head pairs
                    ct = pr
                    av = [mm_ps() for _ in range(2)]          # [128,512] x sq-halves
                    sums = [mm_ps((33, 512)) for _ in range(2)]
                    for sk in range(8):
                        for hh in range(2):   # head within pair
                            head = 2 * pr + hh
                            base = hh * 64
                            prb = prp.tile([128, 1024], bf16, tag="pr", name=_nm("pr"))
                            for half in range(2):
                                sc = mm_ps()
                                nc.tensor.matmul(
                                    sc[:],
                                    kT[ct][base:base + 64, sk * 128:(sk + 1) * 128],
                                    qT[ct][base:base + 64, half * 512:(half + 1) * 512],
                                    start=True, stop=True)
                                nc.scalar.activation(prb[:, half * 512:(half + 1) * 512],
                                                     sc[:], AF.Exp, bias=0.0, scale=0.125)
                            for half in range(2):
                                nc.tensor.matmul(
                                    av[half][base:base + 64, :],
                                    vaug[sk][:, head, 0:64],
                                    prb[:, half * 512:(half + 1) * 512],
                                    start=(sk == 0), stop=(sk == 7),
                                    tile_position=(0, base), skip_group_check=True)
                                nc.tensor.matmul(
                                    sums[half][32 * hh:32 * hh + 1, :],
                                    vaug[sk][:, head, 64:65],
                                    prb[:, half * 512:(half + 1) * 512],
                                    start=(sk == 0), stop=(sk == 7),
                                    tile_position=(0, 32 * hh), skip_group_check=True)
                    # normalize: recip of sums rows, broadcast via ones-matmul, multiply
                    rs = sm.tile([33, 1024], f32r, tag="rs")
                    opair = sm.tile([128, 1024], f32, tag="opair")
                    for half in range(2):
                        nc.vector.reciprocal(rs[:, half * 512:(half + 1) * 512],
                                             sums[half][:])
                        nc.scalar.copy(opair[:, half * 512:(half + 1) * 512], av[half][:])
                    for half in range(2):
                        sl = slice(half * 512, (half + 1) * 512)
                        bc0 = mm_ps()
                        nc.tensor.matmul(bc0[:], ones_sb[0:1, 0:128], rs[0:1, sl],
                                         start=True, stop=True,
                                         tile_position=(0, 0), skip_group_check=True)
                        bc1 = mm_ps()
                        nc.tensor.matmul(bc1[:], ones_sb[32:33, 0:128], rs[32:33, sl],
                                         start=True, stop=True,
                                         tile_position=(32, 0), skip_group_check=True)
                        nc.vector.tensor_tensor(out=attn[ct][0:64, sl],
                                                in0=opair[0:64, sl],
                                                in1=bc0[0:64, :], op=OP.mult)
                        nc.vector.tensor_tensor(out=attn[ct][64:128, sl],
                                                in0=opair[64:128, sl],
                                                in1=bc1[64:128, :], op=OP.mult)
                if debug and blk == 0:
                    for t in range(4):
                        nc.sync.dma_start(dbg["attn0"][t * 128:(t + 1) * 128, :], attn[t][:])

                # proj + residual(y)
                pwT = load_w4(dr[f"at{blk}_pwT"][:], tag="pwT", dtype=bf16)
                pb = load_col(f"at{blk}_pb", tag="pb")
                otag = "tC" if blk == 0 else "tB"
                y2 = [ap_.tile([128, S], f32r, tag=f"{otag}{t}", name=_nm(f"{otag}{t}")) for t in range(4)]
                for co in range(4):
                    for sn in range(2):
                        pp = mm_ps()
                        for ci in range(4):
                            nc.tensor.matmul(pp[:], pwT[:, ci, co * 128:(co + 1) * 128],
                                             attn[ci][:, sn * 512:(sn + 1) * 512],
                                             start=(ci == 0), stop=False,
                                             skip_group_check=True)
                        nc.tensor.matmul(pp[:], ident_sb[:],
                                         y[co][:, sn * 512:(sn + 1) * 512],
                                         start=False, stop=True, skip_group_check=True)
                        nc.scalar.activation(y2[co][:, sn * 512:(sn + 1) * 512],
                                             pp[:], AF.Identity,
                                             bias=pb[:, co:co + 1], scale=1.0)
                if debug:
                    for t in range(4):
                        nc.sync.dma_start(dbg[f"y2{blk}"][t * 128:(t + 1) * 128, :],
                                          y2[t][:].bitcast(f32))
                cur = y2

            # ---------- downsample conv (stride 2) ----------
            pads3 = make_pad(4)
            for t in range(4):
                nc.scalar.copy(pad_interior(pads3[t]), cur[t][:].bitcast(f32))
            pst = conv3x3(pads3, 4, "ds_wT", stride=2)
            dsb = load_col("ds_b", tag="dsb")
            for co in range(4):
                ot = sm.tile([128, 256], f32, tag="obuf", name=_nm("obuf"))
                nc.scalar.activation(ot[:], pst[co][0], AF.Identity,
                                     bias=dsb[:, co:co + 1], scale=1.0)
                nc.sync.dma_start(out_d[co * 128:(co + 1) * 128, :], ot[:])

    nc.compile()
    return nc


# ---------------- host side ----------------

def _col4(v):
    """[512] -> [128, 4] column-per-tile layout."""
    return np.ascontiguousarray(np.asarray(v, np.float32).reshape(4, 128).T)


def _coln(v, nt):
    return np.ascontiguousarray(np.asarray(v, np.float32).reshape(nt, 128).T)


def _convT(w):
    """[cout, cin, 3, 3] -> [9, cin, cout]."""
    w = np.asarray(w, np.float32)
    return np.ascontiguousarray(w.transpose(2, 3, 1, 0).reshape(9, w.shape[1], w.shape[0]))


def prep_inputs(x, temb, params):
    import ml_dtypes
    x = np.asarray(x, np.float32)
    temb = np.asarray(temb, np.float32)
    shared = {}
    for i in range(2):
        rp = params["res"][i]
        cin = C1 if i == 0 else C2
        nt = cin // 128
        shared[f"rb{i}_g1s"] = _coln(rp["n1s"], nt)
        shared[f"rb{i}_g1b"] = _coln(rp["n1b"], nt)
        shared[f"rb{i}_c1wT"] = _convT(rp["c1w"])
        shared[f"rb{i}_cb1"] = _col4(np.asarray(rp["c1b"], np.float32)
                                     + np.asarray(rp["tb"], np.float32))
        shared[f"rb{i}_twT"] = np.ascontiguousarray(
            np.asarray(rp["tw"], np.float32).T).astype(ml_dtypes.bfloat16)
        shared[f"rb{i}_g2s"] = _col4(rp["n2s"])
        shared[f"rb{i}_g2b"] = _col4(rp["n2b"])
        shared[f"rb{i}_c2wT"] = _convT(rp["c2w"])
        cb2 = np.asarray(rp["c2b"], np.float32)
        if i == 0:
            cb2 = cb2 + np.asarray(rp["skb"], np.float32)
        shared[f"rb{i}_cb2"] = _col4(cb2)
        ap = params["attn"][i]
        shared[f"at{i}_ns"] = _col4(ap["ns"])
        shared[f"at{i}_nb"] = _col4(ap["nb"])
        for nm, key in (("qwT", "qw"), ("kwT", "kw"), ("vwT", "vw"), ("pwT", "pw")):
            shared[f"at{i}_{nm}"] = np.ascontiguousarray(
                np.asarray(ap[key], np.float32).T).astype(ml_dtypes.bfloat16)
        shared[f"at{i}_qb"] = _col4(ap["qb"])
        shared[f"at{i}_kb"] = _col4(ap["kb"])
        shared[f"at{i}_vb"] = np.asarray(ap["vb"], np.float32).reshape(1, C2).astype(
            ml_dtypes.bfloat16)
        shared[f"at{i}_pb"] = _col4(ap["pb"])
    shared["sk_wT"] = np.ascontiguousarray(
        np.asarray(params["res"][0]["skw"], np.float32)[:, :, 0, 0].T)
    shared["ds_wT"] = _convT(params["ds"]["w"])
    shared["ds_b"] = _col4(params["ds"]["b"])

    in_maps = []
    for b in range(B):
        m = dict(shared)
        m["x"] = np.ascontiguousarray(x[b].reshape(C1, S))
        m["temb"] = np.ascontiguousarray(temb[b].reshape(4, 128).T)
        in_maps.append(m)
    return in_maps


def kernel(x, temb, params):
    nc = build(debug=bool(int(os.environ.get("BASS_DEBUG_STAGE", "0"))))
    in_maps = prep_inputs(x, temb, params)
    res = bass_utils.run_bass_kernel_spmd(nc, in_maps, core_ids=list(range(B)))
    out = np.stack([r["out"].reshape(C2, 16, 16) for r in res.results])
    return out.astype(np.float32)
